# revision 13
# baseline (speedup 1.0000x reference)
"""Trainium2 Bass kernel for nn_DecoderHeadVDP (variance-propagating decoder
attention head), distributed over 8 NeuronCores.

Sharding: core c handles batch b = c//2 and head-group g = c%2 (8 of 16 heads,
i.e. output columns [512*g, 512*(g+1)) of the Wq projection).  Inputs are
pre-sliced on the host so all 8 cores run one identical NEFF (true SPMD).

Math (per core, per head h; all 1/sqrt(D) scaling pre-folded into the Wq
weight tiles):
  qT     = (Wq_mu x^T) / sqrt(D)            [cols, S]   (transposed layout)
  var_qT = (W1 var_x^T + Wq_var (x^2)^T) / D,  W1 = Wq_var + Wq_mu^2
  aT     = k qT   (scores transposed: [s_j, s_i]), causal (i >= j)
  var_aT = kv2 var_qT + var_k qT2,  kv2 = var_k + k^2
  p_un   = exp(aT) masked causally;  u = p_un^2; w = u*var_aT; pw = p_un*w
  AV stage (stationary = v-side with ones columns, moving = p-tensors):
    M  = sum_j p_un v   ; Z  = sum_j p_un
    A1 = sum_j u vv2    ; A3 = sum_j u var_v      (vv2 = var_v + v^2)
    A2 = sum_j w vv2    ; S' = sum_j w
    A4 = sum_j pw vv2
  out_mean = M/Z + x
  out_var  = (S'/Z^4) A1 + (1/Z^2)(A2 + A3) - (2/Z^3) A4
(all AV outputs land transposed [dh, s_i]; a PE re-transpose + per-partition
Z-power scaling produces the final [s_i, dh] tiles.)
"""

import sys

import numpy as np

if "/opt/trn_rl_repo" not in sys.path:
    sys.path.insert(0, "/opt/trn_rl_repo")

B, S, D, H = 4, 1024, 1024, 16
DH = 64          # head dim
P = 128          # partitions
NHC = 8          # heads per core
CT = 4           # column tiles per core (4 * 128 = 512 cols)
KD = 8           # contraction d-tiles (8 * 128 = 1024)
GCOL = 512       # output columns per core
RD = 32.0        # sqrt(D)

_BUILT = None


def _emit(nc, tc):
    import concourse.bass as bass  # noqa: F401
    import concourse.mybir as mybir
    from concourse.masks import make_identity

    f32 = mybir.dt.float32
    AF = mybir.ActivationFunctionType
    OP = mybir.AluOpType

    x_b = nc.dram_tensor("x_b", [S, D], f32, kind="ExternalInput")
    varx_b = nc.dram_tensor("varx_b", [S, D], f32, kind="ExternalInput")
    x_res_d = nc.dram_tensor("x_res", [S, GCOL], f32, kind="ExternalInput")
    wqmu_d = nc.dram_tensor("wq_mu_g", [GCOL, D], f32, kind="ExternalInput")
    wqvar_d = nc.dram_tensor("wq_var_g", [GCOL, D], f32, kind="ExternalInput")
    k_d = nc.dram_tensor("k_g", [NHC, S, DH], f32, kind="ExternalInput")
    vk_d = nc.dram_tensor("var_k_g", [NHC, S, DH], f32, kind="ExternalInput")
    v_d = nc.dram_tensor("v_g", [NHC, S, DH], f32, kind="ExternalInput")
    vv_d = nc.dram_tensor("var_v_g", [NHC, S, DH], f32, kind="ExternalInput")
    outm_d = nc.dram_tensor("out_mean", [S, GCOL], f32, kind="ExternalOutput")
    outv_d = nc.dram_tensor("out_var", [S, GCOL], f32, kind="ExternalOutput")

    ctx_pools = []

    const = tc.alloc_tile_pool(name="const", bufs=1)
    ctx_pools.append(const)
    ident = const.tile([P, P], f32)
    make_identity(nc, ident)

    # persistent q-side tensors (live through phase B)
    qpool = tc.alloc_tile_pool(name="qpool", bufs=1)
    ctx_pools.append(qpool)
    qT = qpool.tile([P, CT, S], f32)       # [col-in-tile, ct, s]
    qT2 = qpool.tile([P, CT, S], f32)
    var_qT = qpool.tile([P, CT, S], f32)

    # ---------------- phase A: weights + x transposes + Q projection -------
    wqt_pool = tc.alloc_tile_pool(name="wqt", bufs=1)
    WqTmu = wqt_pool.tile([P, KD, CT, P], f32)   # [d-in-tile, dt, ct, col]
    W1T = wqt_pool.tile([P, KD, CT, P], f32)
    WqTvar = wqt_pool.tile([P, KD, CT, P], f32)

    wq_pool = tc.alloc_tile_pool(name="wqnat", bufs=1)
    wq_mu_nat = wq_pool.tile([P, CT, D], f32)    # [col-in-tile, ct, d]
    nc.sync.dma_start(wq_mu_nat, wqmu_d[:].rearrange("(ct p) d -> p ct d", p=P))
    wq_var_nat = wq_pool.tile([P, CT, D], f32)
    nc.sync.dma_start(wq_var_nat, wqvar_d[:].rearrange("(ct p) d -> p ct d", p=P))
    w1_nat = wq_pool.tile([P, CT, D], f32)
    nc.scalar.square(w1_nat, wq_mu_nat)
    nc.vector.tensor_add(w1_nat, w1_nat, wq_var_nat)

    psumT = tc.alloc_tile_pool(name="psumT", bufs=2, space="PSUM")
    for src, dst, sc in (
        (wq_mu_nat, WqTmu, 1.0 / RD),
        (w1_nat, W1T, 1.0 / (RD * RD)),
        (wq_var_nat, WqTvar, 1.0 / (RD * RD)),
    ):
        for dt in range(KD):
            tpw = psumT.tile([P, CT, P], f32, tag="tpw")
            for ct in range(CT):
                nc.tensor.transpose(
                    tpw[:, ct, :], src[:, ct, dt * P:(dt + 1) * P], ident
                )
            nc.scalar.mul(dst[:, dt], tpw, sc)
    psumT.release()
    wq_pool.release()

    # x / var_x transposes
    xt_pool = tc.alloc_tile_pool(name="xtp", bufs=1)
    xT = xt_pool.tile([P, KD, S], f32)     # [d-in-tile, dt, s]
    varxT = xt_pool.tile([P, KD, S], f32)
    stage_pool = tc.alloc_tile_pool(name="xstage", bufs=1)
    psumX = tc.alloc_tile_pool(name="psumX", bufs=2, space="PSUM")
    for idx, (dram, dst) in enumerate(((x_b, xT), (varx_b, varxT))):
        for half in range(2):
            nat = stage_pool.tile([P, 4, D], f32, tag="nat",
                                  name=f"nat{idx}_{half}")
            nc.sync.dma_start(
                nat,
                dram[half * 512:(half + 1) * 512].rearrange(
                    "(st p) d -> p st d", p=P
                ),
            )
            for dt in range(KD):
                tpx = psumX.tile([P, 4, P], f32, tag="tpx",
                                 name=f"tpx{idx}_{half}_{dt}")
                for i in range(4):
                    nc.tensor.transpose(
                        tpx[:, i, :], nat[:, i, dt * P:(dt + 1) * P], ident
                    )
                if idx == 0:
                    nc.scalar.copy(
                        dst[:, dt, half * 512:(half + 1) * 512], tpx
                    )
                else:
                    nc.vector.tensor_copy(
                        dst[:, dt, half * 512:(half + 1) * 512], tpx
                    )
    psumX.release()
    stage_pool.release()

    # Q projection matmuls
    psumQ = tc.alloc_tile_pool(name="psumQ", bufs=1, space="PSUM")
    xt2_pool = tc.alloc_tile_pool(name="xt2p", bufs=2)
    for sc_i in range(2):
        ssl = slice(sc_i * 512, (sc_i + 1) * 512)
        mps = [psumQ.tile([P, 512], f32, tag=f"mps{ct}", name=f"mps{ct}_{sc_i}")
               for ct in range(CT)]
        vps = [psumQ.tile([P, 512], f32, tag=f"vps{ct}", name=f"vps{ct}_{sc_i}")
               for ct in range(CT)]
        for ct in range(CT):
            for dt in range(KD):
                nc.tensor.matmul(
                    mps[ct], WqTmu[:, dt, ct], xT[:, dt, ssl],
                    start=(dt == 0), stop=(dt == KD - 1),
                )
        for dt in range(KD):
            xt2 = xt2_pool.tile([P, 512], f32, tag="xt2", name=f"xt2_{sc_i}_{dt}")
            nc.scalar.square(xt2, xT[:, dt, ssl])
            for ct in range(CT):
                nc.tensor.matmul(
                    vps[ct], W1T[:, dt, ct], varxT[:, dt, ssl],
                    start=(dt == 0), stop=False,
                )
                nc.tensor.matmul(
                    vps[ct], WqTvar[:, dt, ct], xt2,
                    start=False, stop=(dt == KD - 1),
                )
        for ct in range(CT):
            nc.scalar.copy(qT[:, ct, ssl], mps[ct])
            nc.scalar.square(qT2[:, ct, ssl], mps[ct])
            nc.vector.tensor_copy(var_qT[:, ct, ssl], vps[ct])
    xt2_pool.release()
    psumQ.release()
    xt_pool.release()
    wqt_pool.release()

    # ---------------- phase B: per-head attention ---------------------------
    pairp = tc.alloc_tile_pool(name="pairp", bufs=2)
    sbB = tc.alloc_tile_pool(name="sbB", bufs=2)
    psumK = tc.alloc_tile_pool(name="psumK", bufs=1, space="PSUM")
    psumB = tc.alloc_tile_pool(name="psumB", bufs=1, space="PSUM")
    psumS = tc.alloc_tile_pool(name="psumS", bufs=2, space="PSUM")

    for t in range(4):  # head pairs
        kn = pairp.tile([P, KD, 2, DH], f32, tag="kn", name=f"kn{t}")
        vkn = pairp.tile([P, KD, 2, DH], f32, tag="vkn", name=f"vkn{t}")
        for r in range(2):
            nc.sync.dma_start(
                kn[:, :, r, :],
                k_d[2 * t + r].rearrange("(st p) d -> p st d", p=P),
            )
            nc.sync.dma_start(
                vkn[:, :, r, :],
                vk_d[2 * t + r].rearrange("(st p) d -> p st d", p=P),
            )
        kv2n = pairp.tile([P, KD, 2, DH], f32, tag="kv2n", name=f"kv2n{t}", bufs=1)
        nc.scalar.square(kv2n, kn)
        nc.vector.tensor_add(kv2n, kv2n, vkn)

        # KVT: [2*dh, (k|kv2|var_k), s_j] transposed k-side tensors
        KVT = pairp.tile([P, 3, S], f32, tag="KVT", name=f"KVT{t}")
        for st in range(KD):
            tpb = psumK.tile([P, 3, P], f32, tag="tpb", name=f"tpb{t}_{st}")
            for j, src in enumerate((kn, kv2n, vkn)):
                nc.tensor.transpose(tpb[0:64, j, :], src[:, st, 0, :], ident)
                # odd head -> partitions 64:128 (transpose-mode requires
                # base 0, so use a regular matmul against the identity)
                nc.tensor.matmul(
                    tpb[64:128, j, :], src[:, st, 1, :], ident,
                    start=True, stop=True, tile_position=(0, 64),
                )
            nc.vector.tensor_copy(KVT[:, :, st * P:(st + 1) * P], tpb)

        Rs = []
        for r in range(2):
            # [s_j-in-tile, tj, (v | 1 | var_v | vv2 | 1)]
            Rr = pairp.tile([P, KD, 194], f32, tag=f"R{r}", name=f"R{r}_{t}")
            nc.sync.dma_start(
                Rr[:, :, 0:DH],
                v_d[2 * t + r].rearrange("(st p) d -> p st d", p=P),
            )
            nc.sync.dma_start(
                Rr[:, :, 65:129],
                vv_d[2 * t + r].rearrange("(st p) d -> p st d", p=P),
            )
            nc.vector.memset(Rr[:, :, 64:65], 1.0)
            nc.vector.memset(Rr[:, :, 193:194], 1.0)
            nc.scalar.square(Rr[:, :, 129:193], Rr[:, :, 0:DH])
            nc.vector.tensor_add(
                Rr[:, :, 129:193], Rr[:, :, 129:193], Rr[:, :, 65:129]
            )
            Rs.append(Rr)

        for r in range(2):
            h = 2 * t + r
            R = Rs[r]
            pb = 64 * r
            for ic in range(2):
                avt = psumB.tile([P, 4, 512], f32, tag="avt",
                                 name=f"avt{h}_{ic}")
                ntj = 4 * (ic + 1)
                for tj in range(ntj):
                    i0 = max(ic * 512, tj * P)
                    W = (ic + 1) * 512 - i0
                    i0c = i0 - ic * 512
                    jsl = slice(tj * P, (tj + 1) * P)
                    isl = slice(i0, i0 + W)
                    scm = psumS.tile([P, 512], f32, tag="scm",
                                     name=f"scm{h}_{ic}_{tj}")
                    scv = psumS.tile([P, 512], f32, tag="scv", bufs=1,
                                     name=f"scv{h}_{ic}_{tj}")
                    tp = (pb, 0)
                    nc.tensor.matmul(
                        scm[:, 0:W], KVT[pb:pb + 64, 0, jsl],
                        qT[pb:pb + 64, t, isl],
                        start=True, stop=True, tile_position=tp,
                    )
                    nc.tensor.matmul(
                        scv[:, 0:W], KVT[pb:pb + 64, 1, jsl],
                        var_qT[pb:pb + 64, t, isl],
                        start=True, stop=False, tile_position=tp,
                    )
                    nc.tensor.matmul(
                        scv[:, 0:W], KVT[pb:pb + 64, 2, jsl],
                        qT2[pb:pb + 64, t, isl],
                        start=False, stop=True, tile_position=tp,
                    )
                    p_un = sbB.tile([P, 512], f32, tag="p_un", bufs=3,
                                    name=f"pun{h}_{ic}_{tj}")
                    nc.scalar.activation(p_un[:, 0:W], scm[:, 0:W], AF.Exp)
                    if i0 == tj * P:
                        # diagonal tile: zero p_un where i_local < j(partition)
                        nc.gpsimd.affine_select(
                            out=p_un[:, 0:P], in_=p_un[:, 0:P],
                            compare_op=OP.is_ge, fill=0.0, base=0,
                            pattern=[[1, P]], channel_multiplier=-1,
                        )
                    u = sbB.tile([P, 512], f32, tag="u", bufs=3,
                                 name=f"u{h}_{ic}_{tj}")
                    nc.scalar.square(u[:, 0:W], p_un[:, 0:W])
                    w_ = sbB.tile([P, 512], f32, tag="w_", bufs=3,
                                  name=f"w{h}_{ic}_{tj}")
                    nc.vector.tensor_mul(w_[:, 0:W], u[:, 0:W], scv[:, 0:W])
                    pw = sbB.tile([P, 512], f32, tag="pw", bufs=3,
                                  name=f"pw{h}_{ic}_{tj}")
                    nc.vector.tensor_mul(pw[:, 0:W], p_un[:, 0:W], w_[:, 0:W])

                    first = tj == 0
                    last = tj == ntj - 1
                    osl = slice(i0c, i0c + W)
                    nc.tensor.matmul(
                        avt[0:65, 0, osl], R[:, tj, 0:65], p_un[:, 0:W],
                        start=first, stop=last,
                    )
                    nc.tensor.matmul(
                        avt[0:128, 1, osl], R[:, tj, 65:193], u[:, 0:W],
                        start=first, stop=last,
                    )
                    nc.tensor.matmul(
                        avt[0:65, 2, osl], R[:, tj, 129:194], w_[:, 0:W],
                        start=first, stop=last,
                    )
                    nc.tensor.matmul(
                        avt[0:64, 3, osl], R[:, tj, 129:193], pw[:, 0:W],
                        start=first, stop=last,
                    )

                # ---- epilogue for (h, ic) --------------------------------
                ev = sbB.tile([P, 4, 512], f32, tag="ev", name=f"ev{h}_{ic}")
                nc.scalar.copy(ev[0:65, 0, :], avt[0:65, 0, :])
                nc.scalar.copy(ev[0:128, 1, :], avt[:, 1, :])
                nc.vector.tensor_copy(ev[0:65, 2, :], avt[0:65, 2, :])
                nc.vector.tensor_copy(ev[0:64, 3, :], avt[0:64, 3, :])

                # re-transpose into (reused) avt banks:
                #  bank0: [Mt(4x64) | A2t(4x64)]   (DVE reads)
                #  bank1: [A1t(4x64) | A3t(4x64)]  (DVE reads)
                #  bank2: [A4t(4x64) | Zt(4) | S't(4)]  (ACT reads)
                id64 = ident[0:64, 0:64]
                id64h = ident[64:128, 64:128]
                id1h = ident[64:65, 64:65]
                for ti in range(4):
                    csl = slice(ti * 64, ti * 64 + 64)
                    csl2 = slice(256 + ti * 64, 256 + ti * 64 + 64)
                    tisl = slice(ti * P, (ti + 1) * P)
                    nc.tensor.transpose(avt[:, 0, csl], ev[0:64, 0, tisl], id64)
                    nc.tensor.transpose(avt[:, 0, csl2], ev[0:64, 2, tisl], id64)
                    nc.tensor.transpose(avt[:, 1, csl], ev[64:128, 1, tisl], id64h)
                    nc.tensor.transpose(avt[:, 1, csl2], ev[0:64, 1, tisl], id64)
                    nc.tensor.transpose(avt[:, 2, csl], ev[0:64, 3, tisl], id64)
                    nc.tensor.transpose(
                        avt[:, 2, 256 + ti:257 + ti], ev[64:65, 0, tisl], id1h
                    )
                    nc.tensor.transpose(
                        avt[:, 2, 260 + ti:261 + ti], ev[64:65, 2, tisl], id1h
                    )

                # Z' powers: zr rows = [Zr, Zr^2, -2Zr^3, S'*Zr^4] per ti
                zs = sbB.tile([P, 2, 4], f32, tag="zs", name=f"zs{h}_{ic}")
                nc.scalar.copy(
                    zs, avt[:, 2, 256:264].rearrange("p (a b) -> p a b", a=2)
                )
                zr = sbB.tile([P, 4, 4], f32, tag="zr", name=f"zr{h}_{ic}")
                nc.vector.reciprocal(zr[:, 0, :], zs[:, 0, :])
                nc.vector.tensor_mul(zr[:, 1, :], zr[:, 0, :], zr[:, 0, :])
                nc.vector.tensor_mul(zr[:, 2, :], zr[:, 1, :], zr[:, 0, :])
                nc.vector.tensor_scalar_mul(zr[:, 2, :], zr[:, 2, :], -2.0)
                nc.vector.tensor_mul(zr[:, 3, :], zr[:, 1, :], zr[:, 1, :])
                nc.vector.tensor_mul(zr[:, 3, :], zr[:, 3, :], zs[:, 1, :])

                xr = sbB.tile([P, 4, DH], f32, tag="xr", name=f"xr{h}_{ic}")
                nc.sync.dma_start(
                    xr,
                    x_res_d[ic * 512:(ic + 1) * 512, h * 64:(h + 1) * 64]
                    .rearrange("(ti p) d -> p ti d", p=P),
                )
                om = sbB.tile([P, 4, DH], f32, tag="om", name=f"om{h}_{ic}")
                ov = sbB.tile([P, 4, DH], f32, tag="ov", name=f"ov{h}_{ic}")
                for ti in range(4):
                    csl = slice(ti * 64, ti * 64 + 64)
                    csl2 = slice(256 + ti * 64, 256 + ti * 64 + 64)
                    # mean = Mt * Zr + x
                    nc.vector.scalar_tensor_tensor(
                        out=om[:, ti, :], in0=avt[:, 0, csl],
                        scalar=zr[:, 0, ti:ti + 1],
                        in1=xr[:, ti, :],
                        op0=OP.mult, op1=OP.add,
                    )
                    # var = S~*A1 + Zr^2*A2 + Zr^2*A3 - 2Zr^3*A4
                    # (each op reads at most one PSUM operand)
                    rA = sbB.tile([P, DH], f32, tag="rA", bufs=2,
                                  name=f"rA_{h}_{ic}_{ti}")
                    nc.scalar.mul(rA, avt[:, 2, csl], zr[:, 2, ti:ti + 1])
                    rB = sbB.tile([P, DH], f32, tag="rB", bufs=2,
                                  name=f"rB_{h}_{ic}_{ti}")
                    nc.vector.scalar_tensor_tensor(
                        out=rB, in0=avt[:, 1, csl2],
                        scalar=zr[:, 1, ti:ti + 1], in1=rA,
                        op0=OP.mult, op1=OP.add,
                    )
                    rC = sbB.tile([P, DH], f32, tag="rC", bufs=2,
                                  name=f"rC_{h}_{ic}_{ti}")
                    nc.vector.scalar_tensor_tensor(
                        out=rC, in0=avt[:, 0, csl2],
                        scalar=zr[:, 1, ti:ti + 1], in1=rB,
                        op0=OP.mult, op1=OP.add,
                    )
                    nc.vector.scalar_tensor_tensor(
                        out=ov[:, ti, :], in0=avt[:, 1, csl],
                        scalar=zr[:, 3, ti:ti + 1], in1=rC,
                        op0=OP.mult, op1=OP.add,
                    )
                nc.sync.dma_start(
                    outm_d[ic * 512:(ic + 1) * 512, h * 64:(h + 1) * 64]
                    .rearrange("(ti p) d -> p ti d", p=P),
                    om,
                )
                nc.sync.dma_start(
                    outv_d[ic * 512:(ic + 1) * 512, h * 64:(h + 1) * 64]
                    .rearrange("(ti p) d -> p ti d", p=P),
                    ov,
                )

    for pool in (psumS, psumB, psumK, sbB, pairp):
        pool.release()
    for pool in reversed(ctx_pools):
        pool.release()


def build():
    global _BUILT
    if _BUILT is not None:
        return _BUILT
    import concourse.tile as tile
    from concourse import bacc

    nc = bacc.Bacc("TRN2", target_bir_lowering=False, debug=False)
    with tile.TileContext(nc) as tc:
        _emit(nc, tc)
    nc.compile()
    _BUILT = nc
    return nc


def make_in_maps(inputs):
    x = np.ascontiguousarray(np.asarray(inputs["x"], dtype=np.float32))
    var_x = np.ascontiguousarray(np.asarray(inputs["var_x"], dtype=np.float32))
    k = np.ascontiguousarray(np.asarray(inputs["k"], dtype=np.float32))
    var_k = np.ascontiguousarray(np.asarray(inputs["var_k"], dtype=np.float32))
    v = np.ascontiguousarray(np.asarray(inputs["v"], dtype=np.float32))
    var_v = np.ascontiguousarray(np.asarray(inputs["var_v"], dtype=np.float32))
    wq_mu = np.ascontiguousarray(np.asarray(inputs["Wq_mu"], dtype=np.float32))
    wq_var = np.ascontiguousarray(np.asarray(inputs["Wq_var"], dtype=np.float32))
    in_maps = []
    for c in range(8):
        b, g = c // 2, c % 2
        in_maps.append({
            "x_b": x[b],
            "varx_b": var_x[b],
            "x_res": np.ascontiguousarray(x[b][:, g * GCOL:(g + 1) * GCOL]),
            "wq_mu_g": np.ascontiguousarray(wq_mu[g * GCOL:(g + 1) * GCOL]),
            "wq_var_g": np.ascontiguousarray(wq_var[g * GCOL:(g + 1) * GCOL]),
            "k_g": np.ascontiguousarray(k[b, g * NHC:(g + 1) * NHC]),
            "var_k_g": np.ascontiguousarray(var_k[b, g * NHC:(g + 1) * NHC]),
            "v_g": np.ascontiguousarray(v[b, g * NHC:(g + 1) * NHC]),
            "var_v_g": np.ascontiguousarray(var_v[b, g * NHC:(g + 1) * NHC]),
        })
    return in_maps


def assemble(results):
    out_mean = np.empty((B, S, D), np.float32)
    out_var = np.empty((B, S, D), np.float32)
    for c, r in enumerate(results):
        b, g = c // 2, c % 2
        out_mean[b, :, g * GCOL:(g + 1) * GCOL] = r["out_mean"]
        out_var[b, :, g * GCOL:(g + 1) * GCOL] = r["out_var"]
    return out_mean, out_var


def kernel(**inputs):
    from concourse.bass_utils import run_bass_kernel_spmd

    nc = build()
    in_maps = make_in_maps(inputs)
    res = run_bass_kernel_spmd(nc, in_maps, core_ids=list(range(8)))
    return assemble(res.results)


# revision 33
# speedup vs baseline: 6684.2728x; 6684.2728x over previous
"""Trainium2 Bass kernel for nn_DecoderHeadVDP (variance-propagating decoder
attention head), distributed over 8 NeuronCores.

Sharding: core c handles batch b = c//2 and head-group g = c%2 (8 of 16 heads,
i.e. output columns [512*g, 512*(g+1)) of the Wq projection).  Inputs are
pre-sliced on the host so all 8 cores run one identical NEFF (true SPMD).

Math (per core, per head h; all 1/sqrt(D) scaling pre-folded into the Wq
weight tiles):
  qT     = (Wq_mu x^T) / sqrt(D)            [cols, S]   (transposed layout)
  var_qT = (W1 var_x^T + Wq_var (x^2)^T) / D,  W1 = Wq_var + Wq_mu^2
  aT     = k qT   (scores transposed: [s_j, s_i]), causal (i >= j)
  var_aT = kv2 var_qT + var_k qT2,  kv2 = var_k + k^2
  p_un   = exp(aT) masked causally;  u = p_un^2; w = u*var_aT; pw = p_un*w
  AV stage (stationary = v-side with ones columns, moving = p-tensors):
    M  = sum_j p_un v   ; Z  = sum_j p_un
    A1 = sum_j u vv2    ; A3 = sum_j u var_v      (vv2 = var_v + v^2)
    A2 = sum_j w vv2    ; S' = sum_j w
    A4 = sum_j pw vv2
  out_mean = M/Z + x
  out_var  = (S'/Z^4) A1 + (1/Z^2)(A2 + A3) - (2/Z^3) A4
(all AV outputs land transposed [dh, s_i]; a PE re-transpose + per-partition
Z-power scaling produces the final [s_i, dh] tiles.)
"""

import sys

import numpy as np

if "/opt/trn_rl_repo" not in sys.path:
    sys.path.insert(0, "/opt/trn_rl_repo")

B, S, D, H = 4, 1024, 1024, 16
DH = 64          # head dim
P = 128          # partitions
NHC = 8          # heads per core
CT = 4           # column tiles per core (4 * 128 = 512 cols)
KD = 8           # contraction d-tiles (8 * 128 = 1024)
GCOL = 512       # output columns per core
RD = 32.0        # sqrt(D)

_BUILT = None


def _emit(nc, tc):
    import concourse.bass as bass  # noqa: F401
    import concourse.mybir as mybir
    from concourse.masks import make_identity

    f32 = mybir.dt.float32
    AF = mybir.ActivationFunctionType
    OP = mybir.AluOpType

    x_b = nc.dram_tensor("x_b", [S, D], f32, kind="ExternalInput")
    varx_b = nc.dram_tensor("varx_b", [S, D], f32, kind="ExternalInput")
    x_res_d = nc.dram_tensor("x_res", [S, GCOL], f32, kind="ExternalInput")
    wqmu_d = nc.dram_tensor("wq_mu_g", [GCOL, D], f32, kind="ExternalInput")
    wqvar_d = nc.dram_tensor("wq_var_g", [GCOL, D], f32, kind="ExternalInput")
    k_d = nc.dram_tensor("k_g", [NHC, S, DH], f32, kind="ExternalInput")
    vk_d = nc.dram_tensor("var_k_g", [NHC, S, DH], f32, kind="ExternalInput")
    v_d = nc.dram_tensor("v_g", [NHC, S, DH], f32, kind="ExternalInput")
    vv_d = nc.dram_tensor("var_v_g", [NHC, S, DH], f32, kind="ExternalInput")
    outm_d = nc.dram_tensor("out_mean", [S, GCOL], f32, kind="ExternalOutput")
    outv_d = nc.dram_tensor("out_var", [S, GCOL], f32, kind="ExternalOutput")

    ctx_pools = []

    const = tc.alloc_tile_pool(name="const", bufs=1)
    ctx_pools.append(const)
    ident = const.tile([P, P], f32)
    make_identity(nc, ident)

    # persistent q-side tensors (live through phase B)
    qpool = tc.alloc_tile_pool(name="qpool", bufs=1)
    ctx_pools.append(qpool)
    qT = qpool.tile([P, CT, S], f32)       # [col-in-tile, ct, s]
    qT2 = qpool.tile([P, CT, S], f32)
    var_qT = qpool.tile([P, CT, S], f32)

    # ---------------- phase A: weights + x transposes + Q projection -------
    wqt_pool = tc.alloc_tile_pool(name="wqt", bufs=1)
    WqTmu = wqt_pool.tile([P, KD, CT, P], f32)   # [d-in-tile, dt, ct, col]
    W1T = wqt_pool.tile([P, KD, CT, P], f32)
    WqTvar = wqt_pool.tile([P, KD, CT, P], f32)

    wq_pool = tc.alloc_tile_pool(name="wqnat", bufs=1)
    wq_mu_nat = wq_pool.tile([P, CT, D], f32)    # [col-in-tile, ct, d]
    nc.sync.dma_start(wq_mu_nat, wqmu_d[:].rearrange("(ct p) d -> p ct d", p=P))
    wq_var_nat = wq_pool.tile([P, CT, D], f32)
    nc.sync.dma_start(wq_var_nat, wqvar_d[:].rearrange("(ct p) d -> p ct d", p=P))
    w1_nat = wq_pool.tile([P, CT, D], f32)
    nc.scalar.square(w1_nat, wq_mu_nat)
    nc.vector.tensor_add(w1_nat, w1_nat, wq_var_nat)

    psumT = tc.alloc_tile_pool(name="psumT", bufs=2, space="PSUM")
    for src, dst, sc in (
        (wq_mu_nat, WqTmu, 1.0 / RD),
        (w1_nat, W1T, 1.0 / (RD * RD)),
        (wq_var_nat, WqTvar, 1.0 / (RD * RD)),
    ):
        for dt in range(KD):
            tpw = psumT.tile([P, CT, P], f32, tag="tpw")
            for ct in range(CT):
                nc.tensor.transpose(
                    tpw[:, ct, :], src[:, ct, dt * P:(dt + 1) * P], ident
                )
            nc.scalar.mul(dst[:, dt], tpw, sc)
    psumT.release()
    wq_pool.release()

    # x / var_x transposes
    xt_pool = tc.alloc_tile_pool(name="xtp", bufs=1)
    xT = xt_pool.tile([P, KD, S], f32)     # [d-in-tile, dt, s]
    varxT = xt_pool.tile([P, KD, S], f32)
    stage_pool = tc.alloc_tile_pool(name="xstage", bufs=1)
    psumX = tc.alloc_tile_pool(name="psumX", bufs=2, space="PSUM")
    for idx, (dram, dst) in enumerate(((x_b, xT), (varx_b, varxT))):
        for half in range(2):
            nat = stage_pool.tile([P, 4, D], f32, tag="nat",
                                  name=f"nat{idx}_{half}")
            nc.sync.dma_start(
                nat,
                dram[half * 512:(half + 1) * 512].rearrange(
                    "(st p) d -> p st d", p=P
                ),
            )
            for dt in range(KD):
                tpx = psumX.tile([P, 4, P], f32, tag="tpx",
                                 name=f"tpx{idx}_{half}_{dt}")
                for i in range(4):
                    nc.tensor.transpose(
                        tpx[:, i, :], nat[:, i, dt * P:(dt + 1) * P], ident
                    )
                if idx == 0:
                    nc.scalar.copy(
                        dst[:, dt, half * 512:(half + 1) * 512], tpx
                    )
                else:
                    nc.vector.tensor_copy(
                        dst[:, dt, half * 512:(half + 1) * 512], tpx
                    )
    psumX.release()
    stage_pool.release()

    # Q projection matmuls
    psumQ = tc.alloc_tile_pool(name="psumQ", bufs=1, space="PSUM")
    xt2_pool = tc.alloc_tile_pool(name="xt2p", bufs=2)
    for sc_i in range(2):
        ssl = slice(sc_i * 512, (sc_i + 1) * 512)
        mps = [psumQ.tile([P, 512], f32, tag=f"mps{ct}", name=f"mps{ct}_{sc_i}")
               for ct in range(CT)]
        vps = [psumQ.tile([P, 512], f32, tag=f"vps{ct}", name=f"vps{ct}_{sc_i}")
               for ct in range(CT)]
        for ct in range(CT):
            for dt in range(KD):
                nc.tensor.matmul(
                    mps[ct], WqTmu[:, dt, ct], xT[:, dt, ssl],
                    start=(dt == 0), stop=(dt == KD - 1),
                )
        for dt in range(KD):
            xt2 = xt2_pool.tile([P, 512], f32, tag="xt2", name=f"xt2_{sc_i}_{dt}")
            nc.scalar.square(xt2, xT[:, dt, ssl])
            for ct in range(CT):
                nc.tensor.matmul(
                    vps[ct], W1T[:, dt, ct], varxT[:, dt, ssl],
                    start=(dt == 0), stop=False,
                )
                nc.tensor.matmul(
                    vps[ct], WqTvar[:, dt, ct], xt2,
                    start=False, stop=(dt == KD - 1),
                )
        for ct in range(CT):
            nc.scalar.copy(qT[:, ct, ssl], mps[ct])
            nc.scalar.square(qT2[:, ct, ssl], mps[ct])
            nc.vector.tensor_copy(var_qT[:, ct, ssl], vps[ct])
    xt2_pool.release()
    psumQ.release()
    xt_pool.release()
    wqt_pool.release()

    # ---------------- phase B: per-head attention ---------------------------
    pairp = tc.alloc_tile_pool(name="pairp", bufs=2)
    sbB = tc.alloc_tile_pool(name="sbB", bufs=2)
    psumK = tc.alloc_tile_pool(name="psumK", bufs=1, space="PSUM")
    psumB = tc.alloc_tile_pool(name="psumB", bufs=1, space="PSUM")
    psumS = tc.alloc_tile_pool(name="psumS", bufs=2, space="PSUM")

    for t in range(4):  # head pairs
        kn = pairp.tile([P, KD, 2, DH], f32, tag="kn", name=f"kn{t}")
        vkn = pairp.tile([P, KD, 2, DH], f32, tag="vkn", name=f"vkn{t}")
        for r in range(2):
            nc.sync.dma_start(
                kn[:, :, r, :],
                k_d[2 * t + r].rearrange("(st p) d -> p st d", p=P),
            )
            nc.sync.dma_start(
                vkn[:, :, r, :],
                vk_d[2 * t + r].rearrange("(st p) d -> p st d", p=P),
            )
        kv2n = pairp.tile([P, KD, 2, DH], f32, tag="kv2n", name=f"kv2n{t}", bufs=1)
        nc.scalar.square(kv2n, kn)
        nc.vector.tensor_add(kv2n, kv2n, vkn)

        # KVT[:, j, :]: j=0 kT, j=1 kv2T, j=2 var_kT (even head rows 0:64,
        # odd head rows 64:128)
        KVT = pairp.tile([P, 3, S], f32, tag="KVT", name=f"KVT{t}")
        for st in range(KD):
            tpb = psumK.tile([P, 3, P], f32, tag="tpb", name=f"tpb{t}_{st}")
            for j, src in enumerate((kn, kv2n, vkn)):
                nc.tensor.transpose(tpb[0:64, j, :], src[:, st, 0, :], ident)
                # odd head -> partitions 64:128 (transpose-mode requires
                # base 0, so use a regular matmul against the identity)
                nc.tensor.matmul(
                    tpb[64:128, j, :], src[:, st, 1, :], ident,
                    start=True, stop=True, tile_position=(0, 64),
                )
            nc.vector.tensor_copy(KVT[:, :, st * P:(st + 1) * P], tpb)

        Rs = []
        for r in range(2):
            # [s_j-in-tile, tj, (v | 1 | var_v | vv2 | 1)]
            Rr = pairp.tile([P, KD, 194], f32, tag=f"R{r}", name=f"R{r}_{t}")
            nc.sync.dma_start(
                Rr[:, :, 0:DH],
                v_d[2 * t + r].rearrange("(st p) d -> p st d", p=P),
            )
            nc.sync.dma_start(
                Rr[:, :, 65:129],
                vv_d[2 * t + r].rearrange("(st p) d -> p st d", p=P),
            )
            nc.gpsimd.memset(Rr[:, :, 64:65], 1.0)
            nc.gpsimd.memset(Rr[:, :, 193:194], 1.0)
            nc.scalar.square(Rr[:, :, 129:193], Rr[:, :, 0:DH])
            nc.vector.tensor_add(
                Rr[:, :, 129:193], Rr[:, :, 129:193], Rr[:, :, 65:129]
            )
            Rs.append(Rr)

        for r in range(2):
            h = 2 * t + r
            R = Rs[r]
            pb = 64 * r
            for ic in range(2):
                avt = psumB.tile([P, 4, 512], f32, tag="avt",
                                 name=f"avt{h}_{ic}")
                ntj = 4 * (ic + 1)
                for tj in range(ntj):
                    i0 = max(ic * 512, tj * P)
                    W = (ic + 1) * 512 - i0
                    i0c = i0 - ic * 512
                    jsl = slice(tj * P, (tj + 1) * P)
                    isl = slice(i0, i0 + W)
                    scm = psumS.tile([P, 512], f32, tag="scm",
                                     name=f"scm{h}_{ic}_{tj}")
                    scv = psumS.tile([P, 512], f32, tag="scv", bufs=1,
                                     name=f"scv{h}_{ic}_{tj}")
                    tp = (pb, 0)
                    nc.tensor.matmul(
                        scm[:, 0:W], KVT[pb:pb + 64, 0, jsl],
                        qT[pb:pb + 64, t, isl],
                        start=True, stop=True, tile_position=tp,
                    )
                    nc.tensor.matmul(
                        scv[:, 0:W], KVT[pb:pb + 64, 1, jsl],
                        var_qT[pb:pb + 64, t, isl],
                        start=True, stop=False, tile_position=tp,
                    )
                    nc.tensor.matmul(
                        scv[:, 0:W], KVT[pb:pb + 64, 2, jsl],
                        qT2[pb:pb + 64, t, isl],
                        start=False, stop=True, tile_position=tp,
                    )
                    p_un = sbB.tile([P, 512], f32, tag="p_un", bufs=3,
                                    name=f"pun{h}_{ic}_{tj}")
                    nc.scalar.activation(p_un[:, 0:W], scm[:, 0:W], AF.Exp)
                    if i0 == tj * P:
                        # diagonal tile: zero p_un where i_local < j(partition)
                        nc.gpsimd.affine_select(
                            out=p_un[:, 0:P], in_=p_un[:, 0:P],
                            compare_op=OP.is_ge, fill=0.0, base=0,
                            pattern=[[1, P]], channel_multiplier=-1,
                        )
                    u = sbB.tile([P, 512], f32, tag="u", bufs=3,
                                 name=f"u{h}_{ic}_{tj}")
                    nc.scalar.square(u[:, 0:W], p_un[:, 0:W])
                    w_ = sbB.tile([P, 512], f32, tag="w_", bufs=3,
                                  name=f"w{h}_{ic}_{tj}")
                    nc.vector.tensor_mul(w_[:, 0:W], u[:, 0:W], scv[:, 0:W])
                    pw = sbB.tile([P, 512], f32, tag="pw", bufs=3,
                                  name=f"pw{h}_{ic}_{tj}")
                    nc.vector.tensor_mul(pw[:, 0:W], p_un[:, 0:W], w_[:, 0:W])

                    first = tj == 0
                    last = tj == ntj - 1
                    osl = slice(i0c, i0c + W)
                    nc.tensor.matmul(
                        avt[0:65, 0, osl], R[:, tj, 0:65], p_un[:, 0:W],
                        start=first, stop=last,
                    )
                    nc.tensor.matmul(
                        avt[0:128, 1, osl], R[:, tj, 65:193], u[:, 0:W],
                        start=first, stop=last,
                    )
                    nc.tensor.matmul(
                        avt[0:65, 2, osl], R[:, tj, 129:194], w_[:, 0:W],
                        start=first, stop=last,
                    )
                    nc.tensor.matmul(
                        avt[0:64, 3, osl], R[:, tj, 129:193], pw[:, 0:W],
                        start=first, stop=last,
                    )

                # ---- epilogue for (h, ic) --------------------------------
                ev = sbB.tile([P, 4, 512], f32, tag="ev", name=f"ev{h}_{ic}")
                nc.scalar.copy(ev[0:65, 0, :], avt[0:65, 0, :])
                nc.scalar.copy(ev[0:128, 1, :], avt[:, 1, :])
                nc.vector.tensor_copy(ev[0:65, 2, :], avt[0:65, 2, :])
                nc.vector.tensor_copy(ev[0:64, 3, :], avt[0:64, 3, :])

                # re-transpose into (reused) avt banks:
                #  bank0: [Mt(4x64) | A2t(4x64)]   (DVE reads)
                #  bank1: [A1t(4x64) | A3t(4x64)]  (DVE reads)
                #  bank2: [A4t(4x64) | Zt(4) | S't(4)]  (ACT reads)
                id64 = ident[0:64, 0:64]
                id64h = ident[64:128, 64:128]
                id1h = ident[64:65, 64:65]
                for ti in range(4):
                    csl = slice(ti * 64, ti * 64 + 64)
                    csl2 = slice(256 + ti * 64, 256 + ti * 64 + 64)
                    tisl = slice(ti * P, (ti + 1) * P)
                    nc.tensor.transpose(avt[:, 0, csl], ev[0:64, 0, tisl], id64)
                    nc.tensor.transpose(avt[:, 0, csl2], ev[0:64, 2, tisl], id64)
                    nc.tensor.transpose(avt[:, 1, csl], ev[64:128, 1, tisl], id64h)
                    nc.tensor.transpose(avt[:, 1, csl2], ev[0:64, 1, tisl], id64)
                    nc.tensor.transpose(avt[:, 2, csl], ev[0:64, 3, tisl], id64)
                    nc.tensor.transpose(
                        avt[:, 2, 256 + ti:257 + ti], ev[64:65, 0, tisl], id1h
                    )
                    nc.tensor.transpose(
                        avt[:, 2, 260 + ti:261 + ti], ev[64:65, 2, tisl], id1h
                    )

                # Z' powers: zr rows = [Zr, Zr^2, -2Zr^3, S'*Zr^4] per ti
                zs = sbB.tile([P, 2, 4], f32, tag="zs", name=f"zs{h}_{ic}")
                nc.scalar.copy(
                    zs, avt[:, 2, 256:264].rearrange("p (a b) -> p a b", a=2)
                )
                zr = sbB.tile([P, 4, 4], f32, tag="zr", name=f"zr{h}_{ic}")
                nc.vector.reciprocal(zr[:, 0, :], zs[:, 0, :])
                nc.vector.tensor_mul(zr[:, 1, :], zr[:, 0, :], zr[:, 0, :])
                nc.vector.tensor_mul(zr[:, 2, :], zr[:, 1, :], zr[:, 0, :])
                nc.vector.tensor_scalar_mul(zr[:, 2, :], zr[:, 2, :], -2.0)
                nc.vector.tensor_mul(zr[:, 3, :], zr[:, 1, :], zr[:, 1, :])
                nc.vector.tensor_mul(zr[:, 3, :], zr[:, 3, :], zs[:, 1, :])

                xr = sbB.tile([P, 4, DH], f32, tag="xr", name=f"xr{h}_{ic}")
                nc.sync.dma_start(
                    xr,
                    x_res_d[ic * 512:(ic + 1) * 512, h * 64:(h + 1) * 64]
                    .rearrange("(ti p) d -> p ti d", p=P),
                )
                om = sbB.tile([P, 4, DH], f32, tag="om", name=f"om{h}_{ic}")
                ov = sbB.tile([P, 4, DH], f32, tag="ov", name=f"ov{h}_{ic}")
                for ti in range(4):
                    csl = slice(ti * 64, ti * 64 + 64)
                    csl2 = slice(256 + ti * 64, 256 + ti * 64 + 64)
                    # mean = Mt * Zr + x
                    nc.vector.scalar_tensor_tensor(
                        out=om[:, ti, :], in0=avt[:, 0, csl],
                        scalar=zr[:, 0, ti:ti + 1],
                        in1=xr[:, ti, :],
                        op0=OP.mult, op1=OP.add,
                    )
                    # var = S~*A1 + Zr^2*A2 + Zr^2*A3 - 2Zr^3*A4
                    # (each op reads at most one PSUM operand)
                    rA = sbB.tile([P, DH], f32, tag="rA", bufs=2,
                                  name=f"rA_{h}_{ic}_{ti}")
                    nc.scalar.mul(rA, avt[:, 2, csl], zr[:, 2, ti:ti + 1])
                    rB = sbB.tile([P, DH], f32, tag="rB", bufs=2,
                                  name=f"rB_{h}_{ic}_{ti}")
                    nc.vector.scalar_tensor_tensor(
                        out=rB, in0=avt[:, 1, csl2],
                        scalar=zr[:, 1, ti:ti + 1], in1=rA,
                        op0=OP.mult, op1=OP.add,
                    )
                    rC = sbB.tile([P, DH], f32, tag="rC", bufs=2,
                                  name=f"rC_{h}_{ic}_{ti}")
                    nc.vector.scalar_tensor_tensor(
                        out=rC, in0=avt[:, 0, csl2],
                        scalar=zr[:, 1, ti:ti + 1], in1=rB,
                        op0=OP.mult, op1=OP.add,
                    )
                    nc.vector.scalar_tensor_tensor(
                        out=ov[:, ti, :], in0=avt[:, 1, csl],
                        scalar=zr[:, 3, ti:ti + 1], in1=rC,
                        op0=OP.mult, op1=OP.add,
                    )
                nc.sync.dma_start(
                    outm_d[ic * 512:(ic + 1) * 512, h * 64:(h + 1) * 64]
                    .rearrange("(ti p) d -> p ti d", p=P),
                    om,
                )
                nc.sync.dma_start(
                    outv_d[ic * 512:(ic + 1) * 512, h * 64:(h + 1) * 64]
                    .rearrange("(ti p) d -> p ti d", p=P),
                    ov,
                )

    for pool in (psumS, psumB, psumK, sbB, pairp):
        pool.release()
    for pool in reversed(ctx_pools):
        pool.release()


def build():
    global _BUILT
    if _BUILT is not None:
        return _BUILT
    import concourse.tile as tile
    from concourse import bacc

    nc = bacc.Bacc("TRN2", target_bir_lowering=False, debug=False)
    with tile.TileContext(nc) as tc:
        _emit(nc, tc)
    nc.compile()
    _BUILT = nc
    return nc


def make_in_maps(inputs):
    x = np.ascontiguousarray(np.asarray(inputs["x"], dtype=np.float32))
    var_x = np.ascontiguousarray(np.asarray(inputs["var_x"], dtype=np.float32))
    k = np.ascontiguousarray(np.asarray(inputs["k"], dtype=np.float32))
    var_k = np.ascontiguousarray(np.asarray(inputs["var_k"], dtype=np.float32))
    v = np.ascontiguousarray(np.asarray(inputs["v"], dtype=np.float32))
    var_v = np.ascontiguousarray(np.asarray(inputs["var_v"], dtype=np.float32))
    wq_mu = np.ascontiguousarray(np.asarray(inputs["Wq_mu"], dtype=np.float32))
    wq_var = np.ascontiguousarray(np.asarray(inputs["Wq_var"], dtype=np.float32))
    in_maps = []
    for c in range(8):
        b, g = c // 2, c % 2
        in_maps.append({
            "x_b": x[b],
            "varx_b": var_x[b],
            "x_res": np.ascontiguousarray(x[b][:, g * GCOL:(g + 1) * GCOL]),
            "wq_mu_g": np.ascontiguousarray(wq_mu[g * GCOL:(g + 1) * GCOL]),
            "wq_var_g": np.ascontiguousarray(wq_var[g * GCOL:(g + 1) * GCOL]),
            "k_g": np.ascontiguousarray(k[b, g * NHC:(g + 1) * NHC]),
            "var_k_g": np.ascontiguousarray(var_k[b, g * NHC:(g + 1) * NHC]),
            "v_g": np.ascontiguousarray(v[b, g * NHC:(g + 1) * NHC]),
            "var_v_g": np.ascontiguousarray(var_v[b, g * NHC:(g + 1) * NHC]),
        })
    return in_maps


def assemble(results):
    out_mean = np.empty((B, S, D), np.float32)
    out_var = np.empty((B, S, D), np.float32)
    for c, r in enumerate(results):
        b, g = c // 2, c % 2
        out_mean[b, :, g * GCOL:(g + 1) * GCOL] = r["out_mean"]
        out_var[b, :, g * GCOL:(g + 1) * GCOL] = r["out_var"]
    return out_mean, out_var


def kernel(**inputs):
    from concourse.bass_utils import run_bass_kernel_spmd

    nc = build()
    in_maps = make_in_maps(inputs)
    res = run_bass_kernel_spmd(nc, in_maps, core_ids=list(range(8)))
    return assemble(res.results)


# revision 38
# speedup vs baseline: 7784.0593x; 1.1645x over previous
"""Trainium2 Bass kernel for nn_DecoderHeadVDP (variance-propagating decoder
attention head), distributed over 8 NeuronCores.

Sharding: core c handles batch b = c//2 and head-group g = c%2 (8 of 16 heads,
i.e. output columns [512*g, 512*(g+1)) of the Wq projection).  Inputs are
pre-sliced on the host so all 8 cores run one identical NEFF (true SPMD).

Math (per core, per head h; all 1/sqrt(D) scaling pre-folded into the Wq
weight tiles):
  qT     = (Wq_mu x^T) / sqrt(D)            [cols, S]   (transposed layout)
  var_qT = (W1 var_x^T + Wq_var (x^2)^T) / D,  W1 = Wq_var + Wq_mu^2
  aT     = k qT   (scores transposed: [s_j, s_i]), causal (i >= j)
  var_aT = kv2 var_qT + var_k qT2,  kv2 = var_k + k^2
  p_un   = exp(aT) masked causally;  u = p_un^2; w = u*var_aT; pw = p_un*w
  AV stage (stationary = v-side with ones columns, moving = p-tensors):
    M  = sum_j p_un v   ; Z  = sum_j p_un
    A1 = sum_j u vv2    ; A3 = sum_j u var_v      (vv2 = var_v + v^2)
    A2 = sum_j w vv2    ; S' = sum_j w
    A4 = sum_j pw vv2
  out_mean = M/Z + x
  out_var  = (S'/Z^4) A1 + (1/Z^2)(A2 + A3) - (2/Z^3) A4
(all AV outputs land transposed [dh, s_i]; a PE re-transpose + per-partition
Z-power scaling produces the final [s_i, dh] tiles.)
"""

import sys

import numpy as np

if "/opt/trn_rl_repo" not in sys.path:
    sys.path.insert(0, "/opt/trn_rl_repo")

B, S, D, H = 4, 1024, 1024, 16
DH = 64          # head dim
P = 128          # partitions
NHC = 8          # heads per core
CT = 4           # column tiles per core (4 * 128 = 512 cols)
KD = 8           # contraction d-tiles (8 * 128 = 1024)
GCOL = 512       # output columns per core
RD = 32.0        # sqrt(D)

_BUILT = None


def _emit(nc, tc):
    import concourse.bass as bass  # noqa: F401
    import concourse.mybir as mybir
    from concourse.masks import make_identity

    f32 = mybir.dt.float32
    AF = mybir.ActivationFunctionType
    OP = mybir.AluOpType

    x_b = nc.dram_tensor("x_b", [S, D], f32, kind="ExternalInput")
    varx_b = nc.dram_tensor("varx_b", [S, D], f32, kind="ExternalInput")
    x_res_d = nc.dram_tensor("x_res", [S, GCOL], f32, kind="ExternalInput")
    wqmu_d = nc.dram_tensor("wq_mu_g", [GCOL, D], f32, kind="ExternalInput")
    wqvar_d = nc.dram_tensor("wq_var_g", [GCOL, D], f32, kind="ExternalInput")
    k_d = nc.dram_tensor("k_g", [NHC, S, DH], f32, kind="ExternalInput")
    vk_d = nc.dram_tensor("var_k_g", [NHC, S, DH], f32, kind="ExternalInput")
    v_d = nc.dram_tensor("v_g", [NHC, S, DH], f32, kind="ExternalInput")
    vv_d = nc.dram_tensor("var_v_g", [NHC, S, DH], f32, kind="ExternalInput")
    outm_d = nc.dram_tensor("out_mean", [S, GCOL], f32, kind="ExternalOutput")
    outv_d = nc.dram_tensor("out_var", [S, GCOL], f32, kind="ExternalOutput")

    ctx_pools = []

    const = tc.alloc_tile_pool(name="const", bufs=1)
    ctx_pools.append(const)
    ident = const.tile([P, P], f32)
    make_identity(nc, ident)
    # causal keep-mask in transposed coords: cmask[j, i] = 1.0 if i >= j
    cmask = const.tile([P, P], f32)
    nc.gpsimd.memset(cmask, 1.0)
    nc.gpsimd.affine_select(
        out=cmask, in_=cmask, compare_op=mybir.AluOpType.is_ge,
        fill=0.0, base=0, pattern=[[1, P]], channel_multiplier=-1,
    )

    # persistent q-side tensors (live through phase B)
    qpool = tc.alloc_tile_pool(name="qpool", bufs=1)
    ctx_pools.append(qpool)
    qT = qpool.tile([P, CT, S], f32)       # [col-in-tile, ct, s]
    qT2 = qpool.tile([P, CT, S], f32)
    var_qT = qpool.tile([P, CT, S], f32)

    # ---------------- phase A: weights + x transposes + Q projection -------
    wqt_pool = tc.alloc_tile_pool(name="wqt", bufs=1)
    WqTmu = wqt_pool.tile([P, KD, CT, P], f32)   # [d-in-tile, dt, ct, col]
    W1T = wqt_pool.tile([P, KD, CT, P], f32)
    WqTvar = wqt_pool.tile([P, KD, CT, P], f32)

    wq_pool = tc.alloc_tile_pool(name="wqnat", bufs=1)
    wq_mu_nat = wq_pool.tile([P, CT, D], f32)    # [col-in-tile, ct, d]
    nc.sync.dma_start(wq_mu_nat, wqmu_d[:].rearrange("(ct p) d -> p ct d", p=P))
    wq_var_nat = wq_pool.tile([P, CT, D], f32)
    nc.sync.dma_start(wq_var_nat, wqvar_d[:].rearrange("(ct p) d -> p ct d", p=P))
    w1_nat = wq_pool.tile([P, CT, D], f32)
    nc.scalar.square(w1_nat, wq_mu_nat)
    nc.vector.tensor_add(w1_nat, w1_nat, wq_var_nat)

    psumT = tc.alloc_tile_pool(name="psumT", bufs=2, space="PSUM")
    for src, dst, sc in (
        (wq_mu_nat, WqTmu, 1.0 / RD),
        (w1_nat, W1T, 1.0 / (RD * RD)),
        (wq_var_nat, WqTvar, 1.0 / (RD * RD)),
    ):
        for dt in range(KD):
            tpw = psumT.tile([P, CT, P], f32, tag="tpw")
            for ct in range(CT):
                nc.tensor.transpose(
                    tpw[:, ct, :], src[:, ct, dt * P:(dt + 1) * P], ident
                )
            nc.scalar.mul(dst[:, dt], tpw, sc)
    psumT.release()
    wq_pool.release()

    # x / var_x transposes
    xt_pool = tc.alloc_tile_pool(name="xtp", bufs=1)
    xT = xt_pool.tile([P, KD, S], f32)     # [d-in-tile, dt, s]
    varxT = xt_pool.tile([P, KD, S], f32)
    stage_pool = tc.alloc_tile_pool(name="xstage", bufs=1)
    psumX = tc.alloc_tile_pool(name="psumX", bufs=2, space="PSUM")
    for idx, (dram, dst) in enumerate(((x_b, xT), (varx_b, varxT))):
        for half in range(2):
            nat = stage_pool.tile([P, 4, D], f32, tag="nat",
                                  name=f"nat{idx}_{half}")
            nc.sync.dma_start(
                nat,
                dram[half * 512:(half + 1) * 512].rearrange(
                    "(st p) d -> p st d", p=P
                ),
            )
            for dt in range(KD):
                tpx = psumX.tile([P, 4, P], f32, tag="tpx",
                                 name=f"tpx{idx}_{half}_{dt}")
                for i in range(4):
                    nc.tensor.transpose(
                        tpx[:, i, :], nat[:, i, dt * P:(dt + 1) * P], ident
                    )
                if idx == 0:
                    nc.scalar.copy(
                        dst[:, dt, half * 512:(half + 1) * 512], tpx
                    )
                else:
                    nc.vector.tensor_copy(
                        dst[:, dt, half * 512:(half + 1) * 512], tpx
                    )
    psumX.release()
    stage_pool.release()

    # Q projection matmuls
    psumQ = tc.alloc_tile_pool(name="psumQ", bufs=1, space="PSUM")
    xt2_pool = tc.alloc_tile_pool(name="xt2p", bufs=2)
    for sc_i in range(2):
        ssl = slice(sc_i * 512, (sc_i + 1) * 512)
        mps = [psumQ.tile([P, 512], f32, tag=f"mps{ct}", name=f"mps{ct}_{sc_i}")
               for ct in range(CT)]
        vps = [psumQ.tile([P, 512], f32, tag=f"vps{ct}", name=f"vps{ct}_{sc_i}")
               for ct in range(CT)]
        for ct in range(CT):
            for dt in range(KD):
                nc.tensor.matmul(
                    mps[ct], WqTmu[:, dt, ct], xT[:, dt, ssl],
                    start=(dt == 0), stop=(dt == KD - 1),
                )
        for dt in range(KD):
            xt2 = xt2_pool.tile([P, 512], f32, tag="xt2", name=f"xt2_{sc_i}_{dt}")
            nc.scalar.square(xt2, xT[:, dt, ssl])
            for ct in range(CT):
                nc.tensor.matmul(
                    vps[ct], W1T[:, dt, ct], varxT[:, dt, ssl],
                    start=(dt == 0), stop=False,
                )
                nc.tensor.matmul(
                    vps[ct], WqTvar[:, dt, ct], xt2,
                    start=False, stop=(dt == KD - 1),
                )
        for ct in range(CT):
            nc.scalar.copy(qT[:, ct, ssl], mps[ct])
            nc.scalar.square(qT2[:, ct, ssl], mps[ct])
            nc.vector.tensor_copy(var_qT[:, ct, ssl], vps[ct])
    xt2_pool.release()
    psumQ.release()
    xt_pool.release()
    wqt_pool.release()

    # ---------------- phase B: per-head attention ---------------------------
    pairp = tc.alloc_tile_pool(name="pairp", bufs=2)
    sbB = tc.alloc_tile_pool(name="sbB", bufs=2)
    psumK = tc.alloc_tile_pool(name="psumK", bufs=1, space="PSUM")
    psumB = tc.alloc_tile_pool(name="psumB", bufs=1, space="PSUM")
    psumS = tc.alloc_tile_pool(name="psumS", bufs=2, space="PSUM")

    def prep_pair(t):
        kn = pairp.tile([P, KD, 2, DH], f32, tag="kn", name=f"kn{t}")
        vkn = pairp.tile([P, KD, 2, DH], f32, tag="vkn", name=f"vkn{t}")
        for r in range(2):
            nc.sync.dma_start(
                kn[:, :, r, :],
                k_d[2 * t + r].rearrange("(st p) d -> p st d", p=P),
            )
            nc.sync.dma_start(
                vkn[:, :, r, :],
                vk_d[2 * t + r].rearrange("(st p) d -> p st d", p=P),
            )
        kv2n = pairp.tile([P, KD, 2, DH], f32, tag="kv2n", name=f"kv2n{t}", bufs=1)
        nc.scalar.square(kv2n, kn)
        nc.vector.tensor_add(kv2n, kv2n, vkn)

        # KVT[:, j, :]: j=0 kT, j=1 kv2T, j=2 var_kT (even head rows 0:64,
        # odd head rows 64:128)
        KVT = pairp.tile([P, 3, S], f32, tag="KVT", name=f"KVT{t}")
        for st in range(KD):
            tpb = psumK.tile([P, 3, P], f32, tag="tpb", name=f"tpb{t}_{st}")
            for j, src in enumerate((kn, kv2n, vkn)):
                nc.tensor.transpose(tpb[0:64, j, :], src[:, st, 0, :], ident)
                # odd head -> partitions 64:128 (transpose-mode requires
                # base 0, so use a regular matmul against the identity)
                nc.tensor.matmul(
                    tpb[64:128, j, :], src[:, st, 1, :], ident,
                    start=True, stop=True, tile_position=(0, 64),
                )
            nc.vector.tensor_copy(KVT[:, :, st * P:(st + 1) * P], tpb)

        Rs = []
        for r in range(2):
            # [s_j-in-tile, tj, (v | 1 | var_v | vv2 | 1)]
            Rr = pairp.tile([P, KD, 194], f32, tag=f"R{r}", name=f"R{r}_{t}")
            nc.sync.dma_start(
                Rr[:, :, 0:DH],
                v_d[2 * t + r].rearrange("(st p) d -> p st d", p=P),
            )
            nc.sync.dma_start(
                Rr[:, :, 65:129],
                vv_d[2 * t + r].rearrange("(st p) d -> p st d", p=P),
            )
            nc.gpsimd.memset(Rr[:, :, 64:65], 1.0)
            nc.gpsimd.memset(Rr[:, :, 193:194], 1.0)
            nc.scalar.square(Rr[:, :, 129:193], Rr[:, :, 0:DH])
            nc.vector.tensor_add(
                Rr[:, :, 129:193], Rr[:, :, 129:193], Rr[:, :, 65:129]
            )
            Rs.append(Rr)
        return KVT, Rs

    prepped = {0: prep_pair(0)}
    for t in range(4):  # head pairs
        KVT, Rs = prepped.pop(t)
        for r in range(2):
            if r == 1 and t < 3:
                # software-pipeline: emit next pair's prep between heads
                prepped[t + 1] = prep_pair(t + 1)
            h = 2 * t + r
            R = Rs[r]
            pb = 64 * r
            for ic in range(2):
                avt = psumB.tile([P, 4, 512], f32, tag="avt",
                                 name=f"avt{h}_{ic}")
                ntj = 4 * (ic + 1)
                for tj in range(ntj):
                    i0 = max(ic * 512, tj * P)
                    W = (ic + 1) * 512 - i0
                    i0c = i0 - ic * 512
                    jsl = slice(tj * P, (tj + 1) * P)
                    isl = slice(i0, i0 + W)
                    scm = psumS.tile([P, 512], f32, tag="scm",
                                     name=f"scm{h}_{ic}_{tj}")
                    scv = psumS.tile([P, 512], f32, tag="scv", bufs=1,
                                     name=f"scv{h}_{ic}_{tj}")
                    tp = (pb, 0)
                    nc.tensor.matmul(
                        scm[:, 0:W], KVT[pb:pb + 64, 0, jsl],
                        qT[pb:pb + 64, t, isl],
                        start=True, stop=True, tile_position=tp,
                    )
                    nc.tensor.matmul(
                        scv[:, 0:W], KVT[pb:pb + 64, 1, jsl],
                        var_qT[pb:pb + 64, t, isl],
                        start=True, stop=False, tile_position=tp,
                    )
                    nc.tensor.matmul(
                        scv[:, 0:W], KVT[pb:pb + 64, 2, jsl],
                        qT2[pb:pb + 64, t, isl],
                        start=False, stop=True, tile_position=tp,
                    )
                    p_un = sbB.tile([P, 512], f32, tag="p_un", bufs=3,
                                    name=f"pun{h}_{ic}_{tj}")
                    nc.scalar.activation(p_un[:, 0:W], scm[:, 0:W], AF.Exp)
                    if i0 == tj * P:
                        # diagonal tile: zero p_un where i_local < j(partition)
                        nc.vector.tensor_mul(p_un[:, 0:P], p_un[:, 0:P], cmask)
                    u = sbB.tile([P, 512], f32, tag="u", bufs=3,
                                 name=f"u{h}_{ic}_{tj}")
                    nc.scalar.square(u[:, 0:W], p_un[:, 0:W])
                    w_ = sbB.tile([P, 512], f32, tag="w_", bufs=3,
                                  name=f"w{h}_{ic}_{tj}")
                    nc.vector.tensor_mul(w_[:, 0:W], u[:, 0:W], scv[:, 0:W])
                    pw = sbB.tile([P, 512], f32, tag="pw", bufs=3,
                                  name=f"pw{h}_{ic}_{tj}")
                    nc.vector.tensor_mul(pw[:, 0:W], p_un[:, 0:W], w_[:, 0:W])

                    first = tj == 0
                    last = tj == ntj - 1
                    osl = slice(i0c, i0c + W)
                    nc.tensor.matmul(
                        avt[0:65, 0, osl], R[:, tj, 0:65], p_un[:, 0:W],
                        start=first, stop=last,
                    )
                    nc.tensor.matmul(
                        avt[0:128, 1, osl], R[:, tj, 65:193], u[:, 0:W],
                        start=first, stop=last,
                    )
                    nc.tensor.matmul(
                        avt[0:65, 2, osl], R[:, tj, 129:194], w_[:, 0:W],
                        start=first, stop=last,
                    )
                    nc.tensor.matmul(
                        avt[0:64, 3, osl], R[:, tj, 129:193], pw[:, 0:W],
                        start=first, stop=last,
                    )

                # ---- epilogue for (h, ic) --------------------------------
                ev = sbB.tile([P, 4, 512], f32, tag="ev", name=f"ev{h}_{ic}")
                nc.scalar.copy(ev[0:65, 0, :], avt[0:65, 0, :])
                nc.scalar.copy(ev[0:128, 1, :], avt[:, 1, :])
                nc.vector.tensor_copy(ev[0:65, 2, :], avt[0:65, 2, :])
                nc.vector.tensor_copy(ev[0:64, 3, :], avt[0:64, 3, :])

                # re-transpose into (reused) avt banks:
                #  bank0: [Mt(4x64) | A2t(4x64)]   (DVE reads)
                #  bank1: [A1t(4x64) | A3t(4x64)]  (DVE reads)
                #  bank2: [A4t(4x64) | Zt(4) | S't(4)]  (ACT reads)
                id64 = ident[0:64, 0:64]
                id64h = ident[64:128, 64:128]
                id1h = ident[64:65, 64:65]
                for ti in range(4):
                    csl = slice(ti * 64, ti * 64 + 64)
                    csl2 = slice(256 + ti * 64, 256 + ti * 64 + 64)
                    tisl = slice(ti * P, (ti + 1) * P)
                    nc.tensor.transpose(avt[:, 0, csl], ev[0:64, 0, tisl], id64)
                    nc.tensor.transpose(avt[:, 0, csl2], ev[0:64, 2, tisl], id64)
                    nc.tensor.transpose(avt[:, 1, csl], ev[64:128, 1, tisl], id64h)
                    nc.tensor.transpose(avt[:, 1, csl2], ev[0:64, 1, tisl], id64)
                    nc.tensor.transpose(avt[:, 2, csl], ev[0:64, 3, tisl], id64)
                    nc.tensor.transpose(
                        avt[:, 2, 256 + ti:257 + ti], ev[64:65, 0, tisl], id1h
                    )
                    nc.tensor.transpose(
                        avt[:, 2, 260 + ti:261 + ti], ev[64:65, 2, tisl], id1h
                    )

                # single wholesale evac of the transposed banks -> frees avt
                # for the next (h, ic) unit; the combines below read SBUF
                # off the critical path.
                Tsb = sbB.tile([P, 3, 512], f32, tag="Tsb", name=f"Tsb{h}_{ic}")
                nc.scalar.copy(Tsb, avt[:, 0:3, :])

                # Z' powers: zr rows = [Zr, Zr^2, -2Zr^3, S'*Zr^4] per ti
                zr = sbB.tile([P, 4, 4], f32, tag="zr", name=f"zr{h}_{ic}")
                nc.vector.reciprocal(zr[:, 0, :], Tsb[:, 2, 256:260])
                nc.vector.tensor_mul(zr[:, 1, :], zr[:, 0, :], zr[:, 0, :])
                nc.vector.tensor_mul(zr[:, 2, :], zr[:, 1, :], zr[:, 0, :])
                nc.vector.tensor_scalar_mul(zr[:, 2, :], zr[:, 2, :], -2.0)
                nc.vector.tensor_mul(zr[:, 3, :], zr[:, 1, :], zr[:, 1, :])
                nc.vector.tensor_mul(zr[:, 3, :], zr[:, 3, :], Tsb[:, 2, 260:264])

                xr = sbB.tile([P, 4, DH], f32, tag="xr", name=f"xr{h}_{ic}")
                nc.sync.dma_start(
                    xr,
                    x_res_d[ic * 512:(ic + 1) * 512, h * 64:(h + 1) * 64]
                    .rearrange("(ti p) d -> p ti d", p=P),
                )
                om = sbB.tile([P, 4, DH], f32, tag="om", name=f"om{h}_{ic}")
                ov = sbB.tile([P, 4, DH], f32, tag="ov", name=f"ov{h}_{ic}")
                for ti in range(4):
                    csl = slice(ti * 64, ti * 64 + 64)
                    csl2 = slice(256 + ti * 64, 256 + ti * 64 + 64)
                    # mean = Mt * Zr + x
                    nc.vector.scalar_tensor_tensor(
                        out=om[:, ti, :], in0=Tsb[:, 0, csl],
                        scalar=zr[:, 0, ti:ti + 1],
                        in1=xr[:, ti, :],
                        op0=OP.mult, op1=OP.add,
                    )
                    # var = S~*A1 + Zr^2*(A2 + A3) - 2Zr^3*A4
                    rA = sbB.tile([P, DH], f32, tag="rA", bufs=2,
                                  name=f"rA_{h}_{ic}_{ti}")
                    nc.vector.tensor_add(rA, Tsb[:, 0, csl2], Tsb[:, 1, csl2])
                    rB = sbB.tile([P, DH], f32, tag="rB", bufs=2,
                                  name=f"rB_{h}_{ic}_{ti}")
                    nc.vector.tensor_scalar_mul(rB, Tsb[:, 2, csl],
                                                zr[:, 2, ti:ti + 1])
                    rC = sbB.tile([P, DH], f32, tag="rC", bufs=2,
                                  name=f"rC_{h}_{ic}_{ti}")
                    nc.vector.scalar_tensor_tensor(
                        out=rC, in0=rA, scalar=zr[:, 1, ti:ti + 1], in1=rB,
                        op0=OP.mult, op1=OP.add,
                    )
                    nc.vector.scalar_tensor_tensor(
                        out=ov[:, ti, :], in0=Tsb[:, 1, csl],
                        scalar=zr[:, 3, ti:ti + 1], in1=rC,
                        op0=OP.mult, op1=OP.add,
                    )
                nc.sync.dma_start(
                    outm_d[ic * 512:(ic + 1) * 512, h * 64:(h + 1) * 64]
                    .rearrange("(ti p) d -> p ti d", p=P),
                    om,
                )
                nc.sync.dma_start(
                    outv_d[ic * 512:(ic + 1) * 512, h * 64:(h + 1) * 64]
                    .rearrange("(ti p) d -> p ti d", p=P),
                    ov,
                )

    for pool in (psumS, psumB, psumK, sbB, pairp):
        pool.release()
    for pool in reversed(ctx_pools):
        pool.release()


def build():
    global _BUILT
    if _BUILT is not None:
        return _BUILT
    import concourse.tile as tile
    from concourse import bacc

    nc = bacc.Bacc("TRN2", target_bir_lowering=False, debug=False)
    with tile.TileContext(nc) as tc:
        _emit(nc, tc)
    nc.compile()
    _BUILT = nc
    return nc


def make_in_maps(inputs):
    x = np.ascontiguousarray(np.asarray(inputs["x"], dtype=np.float32))
    var_x = np.ascontiguousarray(np.asarray(inputs["var_x"], dtype=np.float32))
    k = np.ascontiguousarray(np.asarray(inputs["k"], dtype=np.float32))
    var_k = np.ascontiguousarray(np.asarray(inputs["var_k"], dtype=np.float32))
    v = np.ascontiguousarray(np.asarray(inputs["v"], dtype=np.float32))
    var_v = np.ascontiguousarray(np.asarray(inputs["var_v"], dtype=np.float32))
    wq_mu = np.ascontiguousarray(np.asarray(inputs["Wq_mu"], dtype=np.float32))
    wq_var = np.ascontiguousarray(np.asarray(inputs["Wq_var"], dtype=np.float32))
    in_maps = []
    for c in range(8):
        b, g = c // 2, c % 2
        in_maps.append({
            "x_b": x[b],
            "varx_b": var_x[b],
            "x_res": np.ascontiguousarray(x[b][:, g * GCOL:(g + 1) * GCOL]),
            "wq_mu_g": np.ascontiguousarray(wq_mu[g * GCOL:(g + 1) * GCOL]),
            "wq_var_g": np.ascontiguousarray(wq_var[g * GCOL:(g + 1) * GCOL]),
            "k_g": np.ascontiguousarray(k[b, g * NHC:(g + 1) * NHC]),
            "var_k_g": np.ascontiguousarray(var_k[b, g * NHC:(g + 1) * NHC]),
            "v_g": np.ascontiguousarray(v[b, g * NHC:(g + 1) * NHC]),
            "var_v_g": np.ascontiguousarray(var_v[b, g * NHC:(g + 1) * NHC]),
        })
    return in_maps


def assemble(results):
    out_mean = np.empty((B, S, D), np.float32)
    out_var = np.empty((B, S, D), np.float32)
    for c, r in enumerate(results):
        b, g = c // 2, c % 2
        out_mean[b, :, g * GCOL:(g + 1) * GCOL] = r["out_mean"]
        out_var[b, :, g * GCOL:(g + 1) * GCOL] = r["out_var"]
    return out_mean, out_var


def kernel(**inputs):
    from concourse.bass_utils import run_bass_kernel_spmd

    nc = build()
    in_maps = make_in_maps(inputs)
    res = run_bass_kernel_spmd(nc, in_maps, core_ids=list(range(8)))
    return assemble(res.results)
